# revision 34
# baseline (speedup 1.0000x reference)
"""BinaryTreeComposer (tree-LSTM node composition) on 8 TRN2 NeuronCores.

Strategy: output-dim (row) tensor-parallel shard of every 2048x2048 weight.
Core d owns rows [256*d, 256*(d+1)) of all 11 matrices.

Two collective-free launches (measured: each ncfw collective costs ~20us
on this runtime plus ~40us launch skew absorbed by the first one, so a
single-launch all-gather design floors at ~110us; two skew-immune
launches total ~60-75us):

  Launch A (per core, J=256 shard):
    m_pre[2,J] = [lh|S ; rh|S](4096) @ [Wh_s|Us_s]^T
    p[2,J]     = [lh ; rh](2048)     @ ma_s^T
      (alpha scaling hoisted past the GEMV: (a*lh)@W = a*(lh@W);
       for k-chunks shared with stage1 the two moving streams are
       concatenated into one N=512 matmul)
    m = tanh(m_pre + b1); e_part[2,1] = rowdot(m, w_s)
    out s1[2, J+1] = [p | e_part]

  Host: ag[16, J+1] = concat(s1 over cores)   (data movement only)

  Launch B (per core):
    alpha[16,1] via two tiny PE matmuls (emask/ones @ e16)
    la/ra[16,J] = tanh(alpha * p_gathered + ma_b)
    PE-transpose la/ra into [128,16] stationary layout
    gates [i|u] and [lf|rf] as two N=512 accumulation groups with the
    bias folded in as a K=1 matmul; c = i*u + lf*lc + rf*rc; h = tanh(c)

All matmul operands bf16 (full PE rate, halves HBM traffic); everything
else f32; PSUM accumulation f32.
"""

import os
import sys

import numpy as np

for _p in ("/opt/trn_rl_repo",):
    if _p not in sys.path and os.path.isdir(_p):
        sys.path.insert(0, _p)

from ml_dtypes import bfloat16  # noqa: E402

MEM = 2048
NCORES = 8
J = MEM // NCORES  # 256
KP = 128  # contraction chunk (partition count)
NCH1 = (2 * MEM) // KP  # 32 stage1 chunks (concat K=4096)
NCH2 = MEM // KP  # 16 stage2 chunks
NCHG = (2 * MEM) // KP  # 32 gate chunks
R2 = 2 * NCORES  # 16 gathered rows

MM_DTYPE = os.environ.get("BTC_MM_DTYPE", "bf16")

_COMPILED = {}
LAST_RESULTS = []


def _ensure_ntff_hook():
    """Make trace=True work under axon: register the NTFF profile hook
    (the image's antenv lacks axon_hooks) and de-fang upload_artifacts
    (no egress in this container)."""
    import types

    try:
        import antenv  # noqa: F401

        if "antenv.axon_hooks" not in sys.modules:
            from trn_agent_boot.trn_boot import _ntff_profile_via_ctypes

            hook = _ntff_profile_via_ctypes("/opt/axon/libaxon_pjrt.so")
            mod = types.ModuleType("antenv.axon_hooks")
            state = {"hook": hook}
            mod.get_axon_ntff_profile_hook = lambda: state["hook"]
            mod.set_axon_ntff_profile_hook = lambda h: state.update(hook=h)
            sys.modules["antenv.axon_hooks"] = mod
            antenv.axon_hooks = mod
    except Exception:
        pass
    try:
        from concourse import bass_utils

        orig = bass_utils.upload_artifacts
        if not getattr(orig, "_btc_safe", False):

            def safe_upload(tmpdir):
                try:
                    return orig(tmpdir)
                except Exception:
                    return str(tmpdir)

            safe_upload._btc_safe = True
            bass_utils.upload_artifacts = safe_upload
    except Exception:
        pass


def _np_mm_dtype():
    return bfloat16 if MM_DTYPE == "bf16" else np.float32


def _mk_nc():
    from concourse import bacc

    return bacc.Bacc(
        "TRN2", target_bir_lowering=False, debug=False, num_devices=NCORES
    )


def _ctx_pools(nc):
    from contextlib import ExitStack

    import concourse.tile as tile

    ctx = ExitStack()
    tc = ctx.enter_context(tile.TileContext(nc))
    sb = ctx.enter_context(tc.tile_pool(name="sb", bufs=1))
    ps = ctx.enter_context(tc.tile_pool(name="ps", bufs=1, space="PSUM"))
    return ctx, tc, sb, ps


def _build_a():
    from concourse import mybir

    f32 = mybir.dt.float32
    mdt = mybir.dt.bfloat16 if MM_DTYPE == "bf16" else mybir.dt.float32
    Tanh = mybir.ActivationFunctionType.Tanh
    add = mybir.AluOpType.add
    mult = mybir.AluOpType.mult

    nc = _mk_nc()
    x1sp_d = nc.dram_tensor("x1sp", [KP, NCH1, 2], mdt, kind="ExternalInput")
    # chunks 0..15: [Wh | ma] side by side (shared lh/rh stationary, N=512)
    w12_d = nc.dram_tensor("w12", [NCH2, KP, 2 * J], mdt, kind="ExternalInput")
    # chunks 16..31: Us part (stationary [S,S], N=256)
    w1r_d = nc.dram_tensor("w1r", [NCH2, KP, J], mdt, kind="ExternalInput")
    b1wr_d = nc.dram_tensor("b1wr", [2, 2 * J], f32, kind="ExternalInput")
    s1_d = nc.dram_tensor("s1", [2, J + 1], f32, kind="ExternalOutput")

    ctx, tc, sb, ps = _ctx_pools(nc)
    with ctx:
        # big streams on sync ring, 512KB blocks for fine-grained MM pacing
        # (gpsimd SWDGE is NOT usable for these: Q7 descriptor generation for
        # large strided blocks costs more than the HWDGE receipt stalls)
        w12tiles = []
        for b in range(4):
            t = sb.tile([KP, 4, 2 * J], mdt, tag=f"w12_{b}")
            nc.sync.dma_start(
                t[:], w12_d.ap()[b * 4 : (b + 1) * 4].transpose([1, 0, 2])
            )
            w12tiles.append(t)
        w1rtiles = []
        for b in range(2):
            t = sb.tile([KP, 8, J], mdt, tag=f"w1r_{b}")
            nc.sync.dma_start(
                t[:], w1r_d.ap()[b * 8 : (b + 1) * 8].transpose([1, 0, 2])
            )
            w1rtiles.append(t)
        # smalls on scalar ring
        x1t = sb.tile([KP, NCH1, 2], mdt, tag="x1t")
        nc.scalar.dma_start(x1t[:], x1sp_d.ap())
        b1wrt = sb.tile([2, 2 * J], f32, tag="b1wrt")
        nc.scalar.dma_start(b1wrt[:], b1wr_d.ap())

        # one PSUM tile [2, 512]: left half m_pre, right half p
        psb = ps.tile([2, 2 * J], f32, tag="psb")
        for c in range(NCH2):
            nc.tensor.matmul(
                psb[:],
                x1t[:, c, :],
                w12tiles[c // 4][:, c % 4, :],
                start=(c == 0),
                stop=False,
                skip_group_check=True,
            )
        for c in range(NCH2):
            nc.tensor.matmul(
                psb[:, 0:J],
                x1t[:, NCH2 + c, :],
                w1rtiles[c // 8][:, c % 8, :],
                start=False,
                stop=(c == NCH2 - 1),
                skip_group_check=True,
            )

        pre1 = sb.tile([2, J], f32, tag="pre1")
        nc.vector.tensor_tensor(pre1[:], psb[:, 0:J], b1wrt[:, 0:J], add)
        m = sb.tile([2, J], f32, tag="m")
        nc.scalar.activation(m[:], pre1[:], Tanh)
        s1p = sb.tile([2, J], f32, tag="s1p")
        nc.vector.tensor_copy(s1p[:], psb[:, J : 2 * J])
        nc.sync.dma_start(s1_d.ap()[:, 0:J], s1p[:])
        scr = sb.tile([2, J], f32, tag="scr")
        nc.vector.tensor_tensor(scr[:], m[:], b1wrt[:, J : 2 * J], mult)
        s1e = sb.tile([2, 1], f32, tag="s1e")
        nc.vector.tensor_reduce(s1e[:], scr[:], mybir.AxisListType.X, add)
        nc.sync.dma_start(s1_d.ap()[:, J : J + 1], s1e[:])

    nc.compile()
    return nc


def _build_b():
    from concourse import mybir

    f32 = mybir.dt.float32
    mdt = mybir.dt.bfloat16 if MM_DTYPE == "bf16" else mybir.dt.float32
    Tanh = mybir.ActivationFunctionType.Tanh
    Sigmoid = mybir.ActivationFunctionType.Sigmoid
    add = mybir.AluOpType.add
    mult = mybir.AluOpType.mult

    nc = _mk_nc()
    # one consolidated f32 small tensor: [ag(pg|e16) | brep | emask | ones]
    SMF = (J + 1) + J + 2 * R2  # 257 + 256 + 32 = 545
    smf_d = nc.dram_tensor("smf", [R2, SMF], f32, kind="ExternalInput")
    # gate weights, grouped [i | u | lf | rf] along the last axis
    wg_d = nc.dram_tensor("wg", [NCHG, KP, 4 * J], mdt, kind="ExternalInput")
    ident_d = nc.dram_tensor("ident", [R2, R2], mdt, kind="ExternalInput")
    bgb_d = nc.dram_tensor("bgb", [1, 4 * J], mdt, kind="ExternalInput")
    lcrc_d = nc.dram_tensor("lcrc", [1, 2 * J], f32, kind="ExternalInput")
    out_d = nc.dram_tensor("out", [1, 2 * J], f32, kind="ExternalOutput")

    ctx, tc, sb, ps = _ctx_pools(nc)
    with ctx:
        # alpha-path inputs: ONE trigger on the scalar ring
        smf = sb.tile([R2, SMF], f32, tag="smf")
        nc.scalar.dma_start(smf[:], smf_d.ap())
        pg = smf[:, 0:J]
        e16 = smf[:, J : J + 1]
        brept = smf[:, J + 1 : J + 1 + J]
        emsk2t = smf[:, J + 1 + J : SMF]
        # odd-shaped smalls on the (idle) gpsimd SWDGE queue
        identt = sb.tile([R2, R2], mdt, tag="identt")
        nc.gpsimd.dma_start(identt[:], ident_d.ap())
        bgbt = sb.tile([1, 4 * J], mdt, tag="bgbt")
        nc.gpsimd.dma_start(bgbt[:], bgb_d.ap())
        lcrct = sb.tile([1, 2 * J], f32, tag="lcrct")
        nc.gpsimd.dma_start(lcrct[:], lcrc_d.ap())
        # gate weights: 2MB blocks, 2 on sync + 2 on scalar (both HWDGE).
        # smf precedes the scalar blocks, so the alpha path isn't queued
        # behind bulk data; splitting hides the per-block receipt stalls.
        wgtiles = []
        for b in range(4):
            t = sb.tile([KP, 8, 4 * J], mdt, tag=f"wg_{b}")
            src = wg_d.ap()[b * 8 : (b + 1) * 8].transpose([1, 0, 2])
            (nc.sync if b < 2 else nc.scalar).dma_start(t[:], src)
            wgtiles.append(t)

        # ---- alphas via two tiny PE matmuls (f32): esel = emask@e, den = ones@e
        eselp = ps.tile([R2, 1], f32, tag="eselp")
        nc.tensor.matmul(eselp[:], emsk2t[:, 0:R2], e16)
        denp = ps.tile([R2, 1], f32, tag="denp")
        nc.tensor.matmul(denp[:], emsk2t[:, R2 : 2 * R2], e16)
        denr = sb.tile([R2, 1], f32, tag="denr")
        nc.vector.reciprocal(denr[:], denp[:])
        alpha = sb.tile([R2, 1], f32, tag="alpha")
        nc.vector.tensor_tensor(alpha[:], eselp[:], denr[:], mult)

        # ---- la/ra: tanh(alpha * p + ma_b) in gathered [16, 256] layout ----
        actin = sb.tile([R2, J], f32, tag="actin")
        nc.vector.scalar_tensor_tensor(actin[:], pg, alpha[:], brept, mult, add)
        laR = sb.tile([R2, J], mdt, tag="laR")
        nc.scalar.activation(laR[:], actin[:], Tanh)

        # ---- PE transpose into stationary layout: T0/T1 [128, 16] ----
        Ts = []
        for h in range(2):
            pt = ps.tile([KP, R2], mdt, tag=f"psT{h}")
            nc.tensor.transpose(pt[:], laR[:, h * KP : (h + 1) * KP], identt[:])
            t = sb.tile([KP, R2], mdt, tag=f"T{h}")
            nc.vector.tensor_copy(t[:], pt[:])
            Ts.append(t)

        def xcat_col(c):
            cc = c % NCH2
            col = 2 * (cc // 2) + (0 if c < NCH2 else 1)
            return Ts[cc % 2][:, col : col + 1]

        # ---- gates: two N=512 groups, bias folded in as a K=1 matmul ----
        one_b = nc.const_aps.aps[(mdt, 1.0)]
        psgA = ps.tile([1, 2 * J], f32, tag="psgA")  # [z_i | z_u]
        psgB = ps.tile([1, 2 * J], f32, tag="psgB")  # [z_lf | z_rf]
        nc.tensor.matmul(
            psgA[:], one_b[0:1, 0:1], bgbt[:, 0 : 2 * J],
            start=True, stop=False, skip_group_check=True,
        )
        nc.tensor.matmul(
            psgB[:], one_b[0:1, 0:1], bgbt[:, 2 * J : 4 * J],
            start=True, stop=False, skip_group_check=True,
        )
        for c in range(NCHG):
            lhs = xcat_col(c)
            wt = wgtiles[c // 8]
            last = c == NCHG - 1
            nc.tensor.matmul(
                psgA[:], lhs, wt[:, c % 8, 0 : 2 * J],
                start=False, stop=last, skip_group_check=True,
            )
            nc.tensor.matmul(
                psgB[:], lhs, wt[:, c % 8, 2 * J : 4 * J],
                start=False, stop=last, skip_group_check=True,
            )

        # ---- gate tail: sA = [i | u], sB = [lf | rf] ----
        sA = sb.tile([1, 2 * J], f32, tag="sA")
        nc.scalar.activation(sA[:, 0:J], psgA[:, 0:J], Sigmoid)
        nc.scalar.activation(sA[:, J : 2 * J], psgA[:, J : 2 * J], Tanh)
        sB = sb.tile([1, 2 * J], f32, tag="sB")
        nc.scalar.activation(sB[:], psgB[:], Sigmoid)

        co = sb.tile([1, 2 * J], f32, tag="co")
        t1 = sb.tile([1, J], f32, tag="t1")
        nc.vector.tensor_tensor(t1[:], sA[:, 0:J], sA[:, J : 2 * J], mult)  # i*u
        t23 = sb.tile([1, 2 * J], f32, tag="t23")
        nc.vector.tensor_tensor(t23[:], sB[:], lcrct[:], mult)  # [lf*lc | rf*rc]
        t4 = sb.tile([1, J], f32, tag="t4")
        nc.vector.tensor_tensor(t4[:], t1[:], t23[:, 0:J], add)
        nc.vector.tensor_tensor(co[:, 0:J], t4[:], t23[:, J : 2 * J], add)  # c
        nc.scalar.activation(co[:, J : 2 * J], co[:, 0:J], Tanh)  # h

        nc.sync.dma_start(out_d.ap(), co[:])

    nc.compile()
    return nc


def _shard_inputs(inp):
    mdt = _np_mm_dtype()
    f32 = np.float32

    def a(x):
        return np.asarray(x, dtype=np.float32)

    lh, rh, S = a(inp["lh"])[0], a(inp["rh"])[0], a(inp["S"])[0]
    lc, rc, w = a(inp["lc"])[0], a(inp["rc"])[0], a(inp["w"])[0]

    xl = np.concatenate([lh, S]).reshape(NCH1, KP).T
    xr = np.concatenate([rh, S]).reshape(NCH1, KP).T
    x1sp = np.stack([xl, xr], axis=-1).astype(mdt)

    emask = np.zeros((R2, R2), np.float32)
    for r in range(R2):
        emask[r, r % 2 :: 2] = 1.0
    emsk2 = np.concatenate([emask, np.ones((R2, R2), np.float32)], axis=1)
    ident = np.eye(R2, dtype=mdt)
    brep = np.repeat(a(inp["ma_b"]).reshape(NCORES, J), 2, axis=0).astype(f32)

    maps_a, maps_b = [], []
    for d in range(NCORES):
        sl = slice(d * J, (d + 1) * J)
        wh_t = a(inp["Wh_w"])[sl].T  # [2048, 256]
        us_t = a(inp["Us_w"])[sl].T
        ma_t = a(inp["ma_w"])[sl].T
        # chunks 0..15: [Wh | ma]; chunks 16..31: Us
        w12 = np.concatenate(
            [wh_t.reshape(NCH2, KP, J), ma_t.reshape(NCH2, KP, J)], axis=2
        ).astype(mdt)  # [16, 128, 512]
        w1r = us_t.reshape(NCH2, KP, J).copy().astype(mdt)
        b1wr = np.concatenate(
            [np.tile((a(inp["Wh_b"]) + a(inp["Us_b"]))[sl], (2, 1)),
             np.tile(w[sl], (2, 1))],
            axis=1,
        ).astype(f32)  # [2, 512]

        # gates grouped [i | u | lf | rf]
        gs = []
        for l, r in (("ilh", "irh"), ("ulh", "urh"), ("lflh", "lfrh"), ("rflh", "rfrh")):
            gs.append(
                np.concatenate([a(inp[l + "_w"])[sl].T, a(inp[r + "_w"])[sl].T], axis=0)
            )
        wg = (
            np.stack(gs, axis=1).reshape(2 * MEM, 4 * J).reshape(NCHG, KP, 4 * J)
        ).astype(mdt)
        bgb = np.concatenate(
            [
                (a(inp["ilh_b"]) + a(inp["irh_b"]))[sl],
                (a(inp["ulh_b"]) + a(inp["urh_b"]))[sl],
                (a(inp["lflh_b"]) + a(inp["lfrh_b"]))[sl],
                (a(inp["rflh_b"]) + a(inp["rfrh_b"]))[sl],
            ]
        ).reshape(1, 4 * J).astype(mdt)
        lcrc = np.concatenate([lc[sl], rc[sl]]).reshape(1, 2 * J).astype(f32)
        maps_a.append({"x1sp": x1sp, "w12": w12, "w1r": w1r, "b1wr": b1wr})
        maps_b.append(
            {
                "wg": wg,
                "_brep": brep,
                "_emsk2": emsk2,
                "ident": ident,
                "bgb": bgb,
                "lcrc": lcrc,
            }
        )
    return maps_a, maps_b


def kernel(**inputs):
    global LAST_RESULTS
    _ensure_ntff_hook()
    from concourse.bass_utils import run_bass_kernel_spmd

    key = MM_DTYPE
    if key not in _COMPILED:
        _COMPILED[key] = (_build_a(), _build_b())
    nc_a, nc_b = _COMPILED[key]

    maps_a, maps_b = _shard_inputs(inputs)
    cores = list(range(NCORES))

    res_a = run_bass_kernel_spmd(nc_a, maps_a, cores)
    ag = np.concatenate(
        [res_a.results[d]["s1"] for d in range(NCORES)], axis=0
    ).astype(np.float32)  # [16, 257] -- pure gather, no host math
    for mb in maps_b:
        smf = np.concatenate([ag, mb.pop("_brep"), mb.pop("_emsk2")], axis=1)
        mb["smf"] = np.ascontiguousarray(smf, dtype=np.float32)

    res_b = run_bass_kernel_spmd(nc_b, maps_b, cores)
    LAST_RESULTS = [res_a, res_b]

    c = np.concatenate([res_b.results[d]["out"][0, 0:J] for d in range(NCORES)])
    h = np.concatenate([res_b.results[d]["out"][0, J : 2 * J] for d in range(NCORES)])
    return (c.reshape(1, MEM).astype(np.float32), h.reshape(1, MEM).astype(np.float32))


# revision 35
# speedup vs baseline: 1.1120x; 1.1120x over previous
"""BinaryTreeComposer (tree-LSTM node composition) on 8 TRN2 NeuronCores.

Strategy: output-dim (row) tensor-parallel shard of every 2048x2048 weight.
Core d owns rows [256*d, 256*(d+1)) of all 11 matrices.

Two collective-free launches (measured: each ncfw collective costs ~20us
on this runtime plus ~40us launch skew absorbed by the first one, so a
single-launch all-gather design floors at ~110us; two skew-immune
launches total ~60-75us):

  Launch A (per core, J=256 shard):
    m_pre[2,J] = [lh|S ; rh|S](4096) @ [Wh_s|Us_s]^T
    p[2,J]     = [lh ; rh](2048)     @ ma_s^T
      (alpha scaling hoisted past the GEMV: (a*lh)@W = a*(lh@W);
       for k-chunks shared with stage1 the two moving streams are
       concatenated into one N=512 matmul)
    m = tanh(m_pre + b1); e_part[2,1] = rowdot(m, w_s)
    out s1[2, J+1] = [p | e_part]

  Host: ag[16, J+1] = concat(s1 over cores)   (data movement only)

  Launch B (per core):
    alpha[16,1] via two tiny PE matmuls (emask/ones @ e16)
    la/ra[16,J] = tanh(alpha * p_gathered + ma_b)
    PE-transpose la/ra into [128,16] stationary layout
    gates [i|u] and [lf|rf] as two N=512 accumulation groups with the
    bias folded in as a K=1 matmul; c = i*u + lf*lc + rf*rc; h = tanh(c)

All matmul operands bf16 (full PE rate, halves HBM traffic); everything
else f32; PSUM accumulation f32.
"""

import os
import sys

import numpy as np

for _p in ("/opt/trn_rl_repo",):
    if _p not in sys.path and os.path.isdir(_p):
        sys.path.insert(0, _p)

from ml_dtypes import bfloat16  # noqa: E402

MEM = 2048
NCORES = 8
J = MEM // NCORES  # 256
KP = 128  # contraction chunk (partition count)
NCH1 = (2 * MEM) // KP  # 32 stage1 chunks (concat K=4096)
NCH2 = MEM // KP  # 16 stage2 chunks
NCHG = (2 * MEM) // KP  # 32 gate chunks
R2 = 2 * NCORES  # 16 gathered rows

MM_DTYPE = os.environ.get("BTC_MM_DTYPE", "bf16")

_COMPILED = {}
LAST_RESULTS = []


def _ensure_ntff_hook():
    """Make trace=True work under axon: register the NTFF profile hook
    (the image's antenv lacks axon_hooks) and de-fang upload_artifacts
    (no egress in this container)."""
    import types

    try:
        import antenv  # noqa: F401

        if "antenv.axon_hooks" not in sys.modules:
            from trn_agent_boot.trn_boot import _ntff_profile_via_ctypes

            hook = _ntff_profile_via_ctypes("/opt/axon/libaxon_pjrt.so")
            mod = types.ModuleType("antenv.axon_hooks")
            state = {"hook": hook}
            mod.get_axon_ntff_profile_hook = lambda: state["hook"]
            mod.set_axon_ntff_profile_hook = lambda h: state.update(hook=h)
            sys.modules["antenv.axon_hooks"] = mod
            antenv.axon_hooks = mod
    except Exception:
        pass
    try:
        from concourse import bass_utils

        orig = bass_utils.upload_artifacts
        if not getattr(orig, "_btc_safe", False):

            def safe_upload(tmpdir):
                try:
                    return orig(tmpdir)
                except Exception:
                    return str(tmpdir)

            safe_upload._btc_safe = True
            bass_utils.upload_artifacts = safe_upload
    except Exception:
        pass


def _np_mm_dtype():
    return bfloat16 if MM_DTYPE == "bf16" else np.float32


def _mk_nc():
    from concourse import bacc

    return bacc.Bacc(
        "TRN2", target_bir_lowering=False, debug=False, num_devices=NCORES
    )


def _ctx_pools(nc):
    from contextlib import ExitStack

    import concourse.tile as tile

    ctx = ExitStack()
    tc = ctx.enter_context(tile.TileContext(nc))
    sb = ctx.enter_context(tc.tile_pool(name="sb", bufs=1))
    ps = ctx.enter_context(tc.tile_pool(name="ps", bufs=1, space="PSUM"))
    return ctx, tc, sb, ps


def _build_a():
    from concourse import mybir

    f32 = mybir.dt.float32
    mdt = mybir.dt.bfloat16 if MM_DTYPE == "bf16" else mybir.dt.float32
    Tanh = mybir.ActivationFunctionType.Tanh
    add = mybir.AluOpType.add
    mult = mybir.AluOpType.mult

    nc = _mk_nc()
    x1sp_d = nc.dram_tensor("x1sp", [KP, NCH1, 2], mdt, kind="ExternalInput")
    # chunks 0..15: [Wh | ma] side by side (shared lh/rh stationary, N=512)
    w12_d = nc.dram_tensor("w12", [NCH2, KP, 2 * J], mdt, kind="ExternalInput")
    # chunks 16..31: Us part (stationary [S,S], N=256)
    w1r_d = nc.dram_tensor("w1r", [NCH2, KP, J], mdt, kind="ExternalInput")
    b1wr_d = nc.dram_tensor("b1wr", [2, 2 * J], f32, kind="ExternalInput")
    s1_d = nc.dram_tensor("s1", [2, J + 1], f32, kind="ExternalOutput")

    ctx, tc, sb, ps = _ctx_pools(nc)
    with ctx:
        # big streams on sync ring, 512KB blocks for fine-grained MM pacing
        # (gpsimd SWDGE is NOT usable for these: Q7 descriptor generation for
        # large strided blocks costs more than the HWDGE receipt stalls)
        w12tiles = []
        for b in range(4):
            t = sb.tile([KP, 4, 2 * J], mdt, tag=f"w12_{b}")
            nc.sync.dma_start(
                t[:], w12_d.ap()[b * 4 : (b + 1) * 4].transpose([1, 0, 2])
            )
            w12tiles.append(t)
        w1rtiles = []
        for b in range(2):
            t = sb.tile([KP, 8, J], mdt, tag=f"w1r_{b}")
            nc.sync.dma_start(
                t[:], w1r_d.ap()[b * 8 : (b + 1) * 8].transpose([1, 0, 2])
            )
            w1rtiles.append(t)
        # smalls on scalar ring
        x1t = sb.tile([KP, NCH1, 2], mdt, tag="x1t")
        nc.scalar.dma_start(x1t[:], x1sp_d.ap())
        b1wrt = sb.tile([2, 2 * J], f32, tag="b1wrt")
        nc.scalar.dma_start(b1wrt[:], b1wr_d.ap())

        # one PSUM tile [2, 512]: left half m_pre, right half p
        psb = ps.tile([2, 2 * J], f32, tag="psb")
        for c in range(NCH2):
            nc.tensor.matmul(
                psb[:],
                x1t[:, c, :],
                w12tiles[c // 4][:, c % 4, :],
                start=(c == 0),
                stop=False,
                skip_group_check=True,
            )
        for c in range(NCH2):
            nc.tensor.matmul(
                psb[:, 0:J],
                x1t[:, NCH2 + c, :],
                w1rtiles[c // 8][:, c % 8, :],
                start=False,
                stop=(c == NCH2 - 1),
                skip_group_check=True,
            )

        pre1 = sb.tile([2, J], f32, tag="pre1")
        nc.vector.tensor_tensor(pre1[:], psb[:, 0:J], b1wrt[:, 0:J], add)
        m = sb.tile([2, J], f32, tag="m")
        nc.scalar.activation(m[:], pre1[:], Tanh)
        s1p = sb.tile([2, J], f32, tag="s1p")
        nc.vector.tensor_copy(s1p[:], psb[:, J : 2 * J])
        nc.sync.dma_start(s1_d.ap()[:, 0:J], s1p[:])
        scr = sb.tile([2, J], f32, tag="scr")
        nc.vector.tensor_tensor(scr[:], m[:], b1wrt[:, J : 2 * J], mult)
        s1e = sb.tile([2, 1], f32, tag="s1e")
        nc.vector.tensor_reduce(s1e[:], scr[:], mybir.AxisListType.X, add)
        nc.sync.dma_start(s1_d.ap()[:, J : J + 1], s1e[:])

    nc.compile()
    return nc


def _build_b():
    from concourse import mybir

    f32 = mybir.dt.float32
    mdt = mybir.dt.bfloat16 if MM_DTYPE == "bf16" else mybir.dt.float32
    Tanh = mybir.ActivationFunctionType.Tanh
    Sigmoid = mybir.ActivationFunctionType.Sigmoid
    add = mybir.AluOpType.add
    mult = mybir.AluOpType.mult

    nc = _mk_nc()
    # one consolidated f32 small tensor: [ag(pg|e16) | brep | emask | ones]
    SMF = (J + 1) + J + 2 * R2  # 257 + 256 + 32 = 545
    smf_d = nc.dram_tensor("smf", [R2, SMF], f32, kind="ExternalInput")
    # gate weights, grouped [i | u | lf | rf] along the last axis
    wg_d = nc.dram_tensor("wg", [NCHG, KP, 4 * J], mdt, kind="ExternalInput")
    ident_d = nc.dram_tensor("ident", [R2, R2], mdt, kind="ExternalInput")
    bgb_d = nc.dram_tensor("bgb", [1, 4 * J], mdt, kind="ExternalInput")
    lcrc_d = nc.dram_tensor("lcrc", [1, 2 * J], f32, kind="ExternalInput")
    out_d = nc.dram_tensor("out", [1, 2 * J], f32, kind="ExternalOutput")

    ctx, tc, sb, ps = _ctx_pools(nc)
    with ctx:
        # alpha-path inputs: ONE trigger on the scalar ring
        smf = sb.tile([R2, SMF], f32, tag="smf")
        nc.scalar.dma_start(smf[:], smf_d.ap())
        pg = smf[:, 0:J]
        e16 = smf[:, J : J + 1]
        brept = smf[:, J + 1 : J + 1 + J]
        emsk2t = smf[:, J + 1 + J : SMF]
        # odd-shaped smalls on the (idle) gpsimd SWDGE queue
        identt = sb.tile([R2, R2], mdt, tag="identt")
        nc.gpsimd.dma_start(identt[:], ident_d.ap())
        bgbt = sb.tile([1, 4 * J], mdt, tag="bgbt")
        nc.gpsimd.dma_start(bgbt[:], bgb_d.ap())
        lcrct = sb.tile([1, 2 * J], f32, tag="lcrct")
        nc.gpsimd.dma_start(lcrct[:], lcrc_d.ap())
        # gate weights: 2MB blocks on the sync ring. (Splitting across the
        # scalar ring was tried and regressed: both HWDGE rings share the 16
        # SDMA engines, and the contention delays the in-order PE consumption
        # more than the hidden per-block receipt stalls save.)
        wgtiles = []
        for b in range(4):
            t = sb.tile([KP, 8, 4 * J], mdt, tag=f"wg_{b}")
            src = wg_d.ap()[b * 8 : (b + 1) * 8].transpose([1, 0, 2])
            nc.sync.dma_start(t[:], src)
            wgtiles.append(t)

        # ---- alphas via two tiny PE matmuls (f32): esel = emask@e, den = ones@e
        eselp = ps.tile([R2, 1], f32, tag="eselp")
        nc.tensor.matmul(eselp[:], emsk2t[:, 0:R2], e16)
        denp = ps.tile([R2, 1], f32, tag="denp")
        nc.tensor.matmul(denp[:], emsk2t[:, R2 : 2 * R2], e16)
        denr = sb.tile([R2, 1], f32, tag="denr")
        nc.vector.reciprocal(denr[:], denp[:])
        alpha = sb.tile([R2, 1], f32, tag="alpha")
        nc.vector.tensor_tensor(alpha[:], eselp[:], denr[:], mult)

        # ---- la/ra: tanh(alpha * p + ma_b) in gathered [16, 256] layout ----
        actin = sb.tile([R2, J], f32, tag="actin")
        nc.vector.scalar_tensor_tensor(actin[:], pg, alpha[:], brept, mult, add)
        laR = sb.tile([R2, J], mdt, tag="laR")
        nc.scalar.activation(laR[:], actin[:], Tanh)

        # ---- PE transpose into stationary layout: T0/T1 [128, 16] ----
        Ts = []
        for h in range(2):
            pt = ps.tile([KP, R2], mdt, tag=f"psT{h}")
            nc.tensor.transpose(pt[:], laR[:, h * KP : (h + 1) * KP], identt[:])
            t = sb.tile([KP, R2], mdt, tag=f"T{h}")
            nc.vector.tensor_copy(t[:], pt[:])
            Ts.append(t)

        def xcat_col(c):
            cc = c % NCH2
            col = 2 * (cc // 2) + (0 if c < NCH2 else 1)
            return Ts[cc % 2][:, col : col + 1]

        # ---- gates: two N=512 groups, bias folded in as a K=1 matmul ----
        one_b = nc.const_aps.aps[(mdt, 1.0)]
        psgA = ps.tile([1, 2 * J], f32, tag="psgA")  # [z_i | z_u]
        psgB = ps.tile([1, 2 * J], f32, tag="psgB")  # [z_lf | z_rf]
        nc.tensor.matmul(
            psgA[:], one_b[0:1, 0:1], bgbt[:, 0 : 2 * J],
            start=True, stop=False, skip_group_check=True,
        )
        nc.tensor.matmul(
            psgB[:], one_b[0:1, 0:1], bgbt[:, 2 * J : 4 * J],
            start=True, stop=False, skip_group_check=True,
        )
        for c in range(NCHG):
            lhs = xcat_col(c)
            wt = wgtiles[c // 8]
            last = c == NCHG - 1
            nc.tensor.matmul(
                psgA[:], lhs, wt[:, c % 8, 0 : 2 * J],
                start=False, stop=last, skip_group_check=True,
            )
            nc.tensor.matmul(
                psgB[:], lhs, wt[:, c % 8, 2 * J : 4 * J],
                start=False, stop=last, skip_group_check=True,
            )

        # ---- gate tail: sA = [i | u], sB = [lf | rf] ----
        sA = sb.tile([1, 2 * J], f32, tag="sA")
        nc.scalar.activation(sA[:, 0:J], psgA[:, 0:J], Sigmoid)
        nc.scalar.activation(sA[:, J : 2 * J], psgA[:, J : 2 * J], Tanh)
        sB = sb.tile([1, 2 * J], f32, tag="sB")
        nc.scalar.activation(sB[:], psgB[:], Sigmoid)

        co = sb.tile([1, 2 * J], f32, tag="co")
        t1 = sb.tile([1, J], f32, tag="t1")
        nc.vector.tensor_tensor(t1[:], sA[:, 0:J], sA[:, J : 2 * J], mult)  # i*u
        t23 = sb.tile([1, 2 * J], f32, tag="t23")
        nc.vector.tensor_tensor(t23[:], sB[:], lcrct[:], mult)  # [lf*lc | rf*rc]
        t4 = sb.tile([1, J], f32, tag="t4")
        nc.vector.tensor_tensor(t4[:], t1[:], t23[:, 0:J], add)
        nc.vector.tensor_tensor(co[:, 0:J], t4[:], t23[:, J : 2 * J], add)  # c
        nc.scalar.activation(co[:, J : 2 * J], co[:, 0:J], Tanh)  # h

        nc.sync.dma_start(out_d.ap(), co[:])

    nc.compile()
    return nc


def _shard_inputs(inp):
    mdt = _np_mm_dtype()
    f32 = np.float32

    def a(x):
        return np.asarray(x, dtype=np.float32)

    lh, rh, S = a(inp["lh"])[0], a(inp["rh"])[0], a(inp["S"])[0]
    lc, rc, w = a(inp["lc"])[0], a(inp["rc"])[0], a(inp["w"])[0]

    xl = np.concatenate([lh, S]).reshape(NCH1, KP).T
    xr = np.concatenate([rh, S]).reshape(NCH1, KP).T
    x1sp = np.stack([xl, xr], axis=-1).astype(mdt)

    emask = np.zeros((R2, R2), np.float32)
    for r in range(R2):
        emask[r, r % 2 :: 2] = 1.0
    emsk2 = np.concatenate([emask, np.ones((R2, R2), np.float32)], axis=1)
    ident = np.eye(R2, dtype=mdt)
    brep = np.repeat(a(inp["ma_b"]).reshape(NCORES, J), 2, axis=0).astype(f32)

    maps_a, maps_b = [], []
    for d in range(NCORES):
        sl = slice(d * J, (d + 1) * J)
        wh_t = a(inp["Wh_w"])[sl].T  # [2048, 256]
        us_t = a(inp["Us_w"])[sl].T
        ma_t = a(inp["ma_w"])[sl].T
        # chunks 0..15: [Wh | ma]; chunks 16..31: Us
        w12 = np.concatenate(
            [wh_t.reshape(NCH2, KP, J), ma_t.reshape(NCH2, KP, J)], axis=2
        ).astype(mdt)  # [16, 128, 512]
        w1r = us_t.reshape(NCH2, KP, J).copy().astype(mdt)
        b1wr = np.concatenate(
            [np.tile((a(inp["Wh_b"]) + a(inp["Us_b"]))[sl], (2, 1)),
             np.tile(w[sl], (2, 1))],
            axis=1,
        ).astype(f32)  # [2, 512]

        # gates grouped [i | u | lf | rf]
        gs = []
        for l, r in (("ilh", "irh"), ("ulh", "urh"), ("lflh", "lfrh"), ("rflh", "rfrh")):
            gs.append(
                np.concatenate([a(inp[l + "_w"])[sl].T, a(inp[r + "_w"])[sl].T], axis=0)
            )
        wg = (
            np.stack(gs, axis=1).reshape(2 * MEM, 4 * J).reshape(NCHG, KP, 4 * J)
        ).astype(mdt)
        bgb = np.concatenate(
            [
                (a(inp["ilh_b"]) + a(inp["irh_b"]))[sl],
                (a(inp["ulh_b"]) + a(inp["urh_b"]))[sl],
                (a(inp["lflh_b"]) + a(inp["lfrh_b"]))[sl],
                (a(inp["rflh_b"]) + a(inp["rfrh_b"]))[sl],
            ]
        ).reshape(1, 4 * J).astype(mdt)
        lcrc = np.concatenate([lc[sl], rc[sl]]).reshape(1, 2 * J).astype(f32)
        maps_a.append({"x1sp": x1sp, "w12": w12, "w1r": w1r, "b1wr": b1wr})
        maps_b.append(
            {
                "wg": wg,
                "_brep": brep,
                "_emsk2": emsk2,
                "ident": ident,
                "bgb": bgb,
                "lcrc": lcrc,
            }
        )
    return maps_a, maps_b


def kernel(**inputs):
    global LAST_RESULTS
    _ensure_ntff_hook()
    from concourse.bass_utils import run_bass_kernel_spmd

    key = MM_DTYPE
    if key not in _COMPILED:
        _COMPILED[key] = (_build_a(), _build_b())
    nc_a, nc_b = _COMPILED[key]

    maps_a, maps_b = _shard_inputs(inputs)
    cores = list(range(NCORES))

    res_a = run_bass_kernel_spmd(nc_a, maps_a, cores)
    ag = np.concatenate(
        [res_a.results[d]["s1"] for d in range(NCORES)], axis=0
    ).astype(np.float32)  # [16, 257] -- pure gather, no host math
    for mb in maps_b:
        smf = np.concatenate([ag, mb.pop("_brep"), mb.pop("_emsk2")], axis=1)
        mb["smf"] = np.ascontiguousarray(smf, dtype=np.float32)

    res_b = run_bass_kernel_spmd(nc_b, maps_b, cores)
    LAST_RESULTS = [res_a, res_b]

    c = np.concatenate([res_b.results[d]["out"][0, 0:J] for d in range(NCORES)])
    h = np.concatenate([res_b.results[d]["out"][0, J : 2 * J] for d in range(NCORES)])
    return (c.reshape(1, MEM).astype(np.float32), h.reshape(1, MEM).astype(np.float32))
